# revision 1
# baseline (speedup 1.0000x reference)
"""YOLO-style detection head decode on 8 Trainium2 NeuronCores.

Input : x [64, 255, 52, 52] f32
Output: [64, 8112, 85] f32  (bbox(4) | conf(1) | cls(80), sigmoid/exp decoded)

Strategy (pure data parallel, 8 batches per core):
  - host packs per-(batch,anchor) slabs [87, 2706] (grid padded 2704->2706):
      rows 0..82 = channels [tx, ty, conf, cls0..79]  (tw/th dropped)
      rows 83/84 = stride*cx / stride*cy grid constants
      rows 85/86 = raw tw / th (read only by the exp staging pass)
    The per-slab device load covers rows 0..84 in one contiguous DMA.
  - ACT sigmoid on rows 0..82.
  - exp(tw/th + ln(anchor_px)) for all 24 slabs packed into ONE [48, 2706]
    ACT Exp op (anchor scale folded into the bias, raw rows fetched by a
    single 4-level-AP DMA), distributed back into slab rows 85/86 by
    SBUF->SBUF DMA on the otherwise-idle gpsimd engine.
  - transpose to output layout fused into PE matmuls: lhsT = 87 slab rows,
    rhs = constant [87, 85] matrix (stride scale on tx/ty, cx/cy rows ->
    cols 0/1, exp rows -> cols 2/3, data rows -> cols 4..84).  lhsT free dim
    strided by 22 so each of 123 output partitions holds 22 consecutive
    output rows -> 7480 B contiguous DMA store runs.
  - PSUM drained via 3 wide copies/slab (DVE/ACT/DVE) from 2-bank psum tiles.
"""

import numpy as np

G = 52
GG = G * G  # 2704
A = 3
NCH = 85  # 5 + 80
B = 64
N_CORES = 8
B_PER_CORE = B // N_CORES  # 8
STRIDE = 8.0  # 416 / 52
ANCHORS_PX = np.array([[10.0, 13.0], [16.0, 30.0], [33.0, 23.0]], dtype=np.float32)
K_MM = 87  # 83 sigmoid rows + 2 cxcy + 2 exp
R = 22  # output rows per partition
P_OUT = 123  # output partitions per matmul (123*22 = 2706 >= 2704)
FREE = P_OUT * R  # 2706
N_SLABS = B_PER_CORE * A  # 24

_CACHE = {}

# channel order of the 83 sigmoid rows: tx, ty, conf, cls0..cls79
DATA_CH = np.array([0, 1] + list(range(4, NCH)), dtype=np.int64)


def _build_consts():
    g = np.arange(GG, dtype=np.float32)
    cx = STRIDE * (g % G)
    cy = STRIDE * (g // G)
    cxcy = np.zeros((2, FREE), dtype=np.float32)
    cxcy[0, :GG] = cx
    cxcy[1, :GG] = cy

    mmat = np.zeros((K_MM, NCH), dtype=np.float32)
    mmat[0, 0] = STRIDE  # sigmoid(tx) -> col 0, scaled
    mmat[1, 1] = STRIDE  # sigmoid(ty) -> col 1, scaled
    for k in range(81):  # conf + cls -> cols 4..84
        mmat[2 + k, 4 + k] = 1.0
    mmat[83, 0] = 1.0  # stride*cx row -> col 0
    mmat[84, 1] = 1.0  # stride*cy row -> col 1
    mmat[85, 2] = 1.0  # exp(tw)*aw row -> col 2
    mmat[86, 3] = 1.0  # exp(th)*ah row -> col 3

    ebias = np.zeros((2 * N_SLABS, 1), dtype=np.float32)
    for b in range(B_PER_CORE):
        for a in range(A):
            s = A * b + a
            ebias[2 * s + 0, 0] = np.log(ANCHORS_PX[a, 0])
            ebias[2 * s + 1, 0] = np.log(ANCHORS_PX[a, 1])
    return cxcy, mmat, ebias


def build_nc():
    if "nc" in _CACHE:
        return _CACHE["nc"]
    from contextlib import ExitStack

    import concourse.bacc as bacc
    import concourse.tile as tile
    from concourse import mybir
    from concourse.tile_rust import add_dep_helper

    AF = mybir.ActivationFunctionType
    dt = mybir.dt

    nc = bacc.Bacc("TRN2", target_bir_lowering=False, debug=False)
    xe_t = nc.dram_tensor(
        "xe", [B_PER_CORE, A, K_MM, FREE], dt.float32, kind="ExternalInput"
    )
    mmat_t = nc.dram_tensor("mmat", [K_MM, NCH], dt.float32, kind="ExternalInput")
    ebias_t = nc.dram_tensor(
        "ebias", [2 * N_SLABS, 1], dt.float32, kind="ExternalInput"
    )
    out_t = nc.dram_tensor(
        "out", [B_PER_CORE, A, GG, NCH], dt.float32, kind="ExternalOutput"
    )
    xe_ap = xe_t.ap()
    mmat_ap = mmat_t.ap()
    ebias_ap = ebias_t.ap()
    out_ap = out_t.ap()

    with ExitStack() as ctx:
        tc = ctx.enter_context(tile.TileContext(nc))
        singles = ctx.enter_context(tc.tile_pool(name="singles", bufs=1))
        slabs = ctx.enter_context(tc.tile_pool(name="slabs", bufs=4))
        outs = ctx.enter_context(tc.tile_pool(name="outs", bufs=4))
        psums = ctx.enter_context(tc.tile_pool(name="psum", bufs=4, space="PSUM"))

        # preload both ACT LUT table sets at t~0 (sigmoid first, exp last so
        # the real exp below needs no reload); otherwise the first table load
        # serializes in front of the exp->sigmoid chain during pipeline fill
        dummy = singles.tile([1, 2], dt.float32)
        nc.vector.memset(dummy[:, :], 0.0)
        nc.scalar.activation(dummy[:, 1:2], dummy[:, 1:2], AF.Sigmoid)
        nc.scalar.activation(dummy[:, 0:1], dummy[:, 0:1], AF.Exp)

        # one 4-level-AP DMA loads every slab's raw tw/th rows at once, so
        # the exp op (and the first slab loads behind it on the SP sequencer)
        # aren't gated on a chain of small transfers
        staging = singles.tile([2 * N_SLABS, FREE], dt.float32)
        ebias_sb = singles.tile([2 * N_SLABS, 1], dt.float32)
        mmat_sb = singles.tile([K_MM, NCH], dt.float32)
        nc.sync.dma_start(out=staging[:, :], in_=xe_ap[:, :, 85:87, :])
        nc.sync.dma_start(out=ebias_sb[:, :], in_=ebias_ap[:, :])
        nc.sync.dma_start(out=mmat_sb[:, :], in_=mmat_ap[:, :])
        nc.scalar.activation(
            staging[:, :], staging[:, :], AF.Exp, bias=ebias_sb[:, :]
        )

        # warm the PE (HAM + pipeline) with throwaway matmuls on the constant
        # matrix while the first slab loads stream in
        wps = psums.tile([P_OUT, 2, 512], dt.float32, tag="ps")
        for _ in range(16):
            nc.tensor.matmul(
                wps[0:NCH, 0, 0:NCH], mmat_sb[:, :], mmat_sb[:, :],
                start=True, stop=True,
            )

        s2s0 = None
        for b in range(B_PER_CORE):
            for a in range(A):
                s = A * b + a
                slab = slabs.tile([K_MM, FREE], dt.float32)
                # exp rows move by DMA (engine copies need 32-aligned partition
                # bases); issued before the load so the transfer isn't queued
                # behind it
                s2s_i = nc.gpsimd.dma_start(
                    out=slab[85:87, :], in_=staging[2 * s : 2 * s + 2, :]
                )
                if s == 0:
                    s2s0 = s2s_i
                load_i = nc.sync.dma_start(
                    out=slab[0:85, :], in_=xe_ap[b, a, 0:85, :]
                )
                if s == 2:
                    # during pipeline fill, let slab0's tiny exp-row transfer
                    # reach the DMA engines before this load occupies them --
                    # otherwise slab0's matmuls (and the whole slab-buffer
                    # rotation behind them) wait for 4 queued 2.5us loads
                    add_dep_helper(
                        load_i.ins, s2s0.ins, sync=True,
                        reason="pipeline fill: exp-row transfer before load2",
                    )
                nc.scalar.activation(slab[0:83, :], slab[0:83, :], AF.Sigmoid)
                # [K_MM, P_OUT, R]: free index (p, t) -> grid row R*p + t
                slab_r = slab[:, :].rearrange("k (p t) -> k p t", t=R)

                out_sb = outs.tile([P_OUT, R * NCH], dt.float32)
                for pair in range(2):
                    ps = psums.tile([P_OUT, 2, 512], dt.float32, tag="ps")
                    for j in range(12):
                        t = 12 * pair + j
                        if t >= R:
                            break
                        # full 123 partitions even for t>=20: pad cols of xe
                        # are zero, so the 2 out-of-range grid rows compute
                        # to benign zeros (excluded from the store DMAs)
                        bank, jj = divmod(j, 6)
                        nc.tensor.matmul(
                            ps[:, bank, jj * NCH : (jj + 1) * NCH],
                            slab_r[:, :, t],
                            mmat_sb[:, :],
                            start=True,
                            stop=True,
                        )
                    if pair == 0:
                        # t0..11 -> cols 0:1020 in one 2-level-AP copy
                        nc.vector.tensor_copy(
                            out_sb[:, 0 : 12 * NCH].rearrange(
                                "p (k c) -> p k c", k=2
                            ),
                            ps[:, :, 0 : 6 * NCH],
                        )
                    else:
                        nc.vector.tensor_copy(
                            out_sb[:, 12 * NCH : 18 * NCH], ps[:, 0, 0 : 6 * NCH]
                        )
                        # last copy on ACT: in-order with the store DMA below,
                        # so the store issues with no cross-engine wait
                        nc.scalar.copy(
                            out_sb[:, 18 * NCH : 22 * NCH], ps[:, 1, 0 : 4 * NCH]
                        )
                full = (P_OUT - 1) * R  # 2684 rows with a full partition
                if s >= N_SLABS - 2:
                    # split the last slabs' stores so the first column group
                    # ships as soon as its copy lands -> shorter drain tail
                    fr = out_ap[b, a, 0:full, :].rearrange(
                        "(p r) c -> p (r c)", r=R
                    )
                    nc.scalar.dma_start(
                        out=fr[:, 0 : 12 * NCH], in_=out_sb[0 : P_OUT - 1, 0 : 12 * NCH]
                    )
                    nc.scalar.dma_start(
                        out=fr[:, 12 * NCH :], in_=out_sb[0 : P_OUT - 1, 12 * NCH :]
                    )
                else:
                    nc.scalar.dma_start(
                        out=out_ap[b, a, 0:full, :],
                        in_=out_sb[0 : P_OUT - 1, :],
                    )
                nc.scalar.dma_start(
                    out=out_ap[b, a, full:GG, :],
                    in_=out_sb[P_OUT - 1 : P_OUT, 0 : (GG - full) * NCH],
                )

    nc.compile()
    _CACHE["nc"] = nc
    return nc


def _pack_core_input(x_core):
    """x_core [B_PER_CORE, 255, 52, 52] -> xe [B_PER_CORE, A, 87, FREE]."""
    cxcy, _, _ = _build_consts()
    xr = x_core.reshape(B_PER_CORE, A, NCH, GG)
    xe = np.zeros((B_PER_CORE, A, K_MM, FREE), dtype=np.float32)
    xe[:, :, 0:83, 0:GG] = xr[:, :, DATA_CH, :]
    xe[:, :, 83:85, :] = cxcy[None, None]
    xe[:, :, 85:87, 0:GG] = xr[:, :, 2:4, :]
    return xe


def kernel(x):
    x = np.ascontiguousarray(np.asarray(x), dtype=np.float32)
    assert x.shape == (B, A * NCH, G, G), x.shape
    nc = build_nc()
    from concourse.bass_utils import run_bass_kernel_spmd

    _, mmat, ebias = _build_consts()
    in_maps = []
    for c in range(N_CORES):
        in_maps.append(
            {
                "xe": _pack_core_input(x[c * B_PER_CORE : (c + 1) * B_PER_CORE]),
                "mmat": mmat,
                "ebias": ebias,
            }
        )
    # transient NRT_EXEC_UNIT_UNRECOVERABLE has been observed once on a cold
    # first execution and never again; retry a couple of times before failing
    for attempt in range(3):
        try:
            res = run_bass_kernel_spmd(nc, in_maps, core_ids=list(range(N_CORES)))
            break
        except Exception:  # noqa: BLE001
            if attempt == 2:
                raise
            import time

            time.sleep(2.0 * (attempt + 1))
    _CACHE["last_res"] = res
    out = np.concatenate([r["out"] for r in res.results], axis=0)
    return out.reshape(B, A * GG, NCH)



# revision 2
# speedup vs baseline: 1.9565x; 1.9565x over previous
"""YOLO-style detection head decode on 8 Trainium2 NeuronCores.

Input : x [64, 255, 52, 52] f32
Output: [64, 8112, 85] f32  (bbox(4) | conf(1) | cls(80), sigmoid/exp decoded)

Strategy (pure data parallel, 8 batches per core; fp16 transfer dtype —
the 2e-2 rel-err budget admits ~5e-3 worst-case fp16 decode error):
  - host transposes each (batch, anchor) slab to grid-major [2704, 85]
    (channel order tx,ty,tw,th,conf,cls already matches the output), pads
    rows 2704->2706 and tiles as [123 partitions, 22 rows x 85 ch] fp16.
    The device then needs NO transpose: no PE matmul, no PSUM drain.
  - device per slab: one 123-descriptor load (3740 B/descriptor), three
    in-place ACT sigmoids (cols 0:4, cols 2:4 with scale=-1 into scratch,
    cols 4:85), tiny DVE fixups, one 123-descriptor store.
  - exp(t) is computed as sigmoid(t)/sigmoid(-t) so the ACT engine only
    ever needs the sigmoid table: no per-slab 1283 ns ACT-table reloads
    (no hw table set holds both Sigmoid and Exp).
  - box decode folded into two constant tiles: out[0:4] *= [8,8,aw,ah]
    (stride scale + anchor scale), out[0:2] += [8*cx, 8*cy].
  - everything is elementwise in-place on one tile, so DMA traffic is the
    bare input+output (2 x 11.04 MB/core fp16): the shared DMA engine pool
    (360 GB/s) is the roofline at ~62 us; ACT sits at ~47 us under it.
"""

import numpy as np

G = 52
GG = G * G  # 2704
A = 3
NCH = 85  # 5 + 80
B = 64
N_CORES = 8
B_PER_CORE = B // N_CORES  # 8
STRIDE = 8.0  # 416 / 52
ANCHORS_PX = np.array([[10.0, 13.0], [16.0, 30.0], [33.0, 23.0]], dtype=np.float64)
P = 123  # partitions per slab tile
RB = 22  # grid rows per partition
ROWS_PAD = P * RB  # 2706
FREE = RB * NCH  # 1870
N_SLABS = B_PER_CORE * A  # 24
KC_W = A * RB * 4 + RB * 2  # 3 kmul tiles + kadd = 308

_CACHE = {}


def _build_consts():
    """kc [P, KC_W] fp16: per-anchor [8,8,aw,ah] mul tiles + [8cx,8cy] add."""
    g = np.arange(ROWS_PAD, dtype=np.float64)
    cx8 = (STRIDE * (g % G)).reshape(P, RB)
    cy8 = (STRIDE * ((g // G) % G)).reshape(P, RB)  # pad rows wrap; sliced off
    kadd = np.stack([cx8, cy8], axis=-1)  # [P, RB, 2]
    km = np.zeros((A, P, RB, 4), dtype=np.float64)
    km[..., 0] = STRIDE
    km[..., 1] = STRIDE
    for a in range(A):
        km[a, ..., 2] = ANCHORS_PX[a, 0]
        km[a, ..., 3] = ANCHORS_PX[a, 1]
    kc = np.concatenate(
        [km.transpose(1, 0, 2, 3).reshape(P, A * RB * 4), kadd.reshape(P, RB * 2)],
        axis=1,
    )
    return kc.astype(np.float16)


def build_nc():
    if "nc" in _CACHE:
        return _CACHE["nc"]
    from contextlib import ExitStack

    import concourse.bacc as bacc
    import concourse.tile as tile
    from concourse import mybir

    AF = mybir.ActivationFunctionType
    ALU = mybir.AluOpType
    dt = mybir.dt

    nc = bacc.Bacc("TRN2", target_bir_lowering=False, debug=False)
    xe_t = nc.dram_tensor("xe", [N_SLABS, P, FREE], dt.float16, kind="ExternalInput")
    kc_t = nc.dram_tensor("kc", [P, KC_W], dt.float16, kind="ExternalInput")
    out_t = nc.dram_tensor("out", [N_SLABS, P, FREE], dt.float16, kind="ExternalOutput")
    xe_ap = xe_t.ap()
    kc_ap = kc_t.ap()
    out_ap = out_t.ap()

    with ExitStack() as ctx:
        tc = ctx.enter_context(tile.TileContext(nc))
        singles = ctx.enter_context(tc.tile_pool(name="singles", bufs=1))
        slabs = ctx.enter_context(tc.tile_pool(name="slabs", bufs=6))
        scrs = ctx.enter_context(tc.tile_pool(name="scrs", bufs=6))

        kc = singles.tile([P, KC_W], dt.float16)
        nc.sync.dma_start(out=kc[:, :], in_=kc_ap[:, :])
        kmul = [
            kc[:, a * RB * 4 : (a + 1) * RB * 4].rearrange("p (t c) -> p t c", c=4)
            for a in range(A)
        ]
        kadd = kc[:, A * RB * 4 :].rearrange("p (t c) -> p t c", c=2)

        # preload the sigmoid table once; every activation below is Sigmoid,
        # so the ACT engine never reloads a table mid-stream
        dummy = singles.tile([1, 1], dt.float32)
        nc.vector.memset(dummy[:, :], 0.0)
        nc.scalar.activation(dummy[:, :], dummy[:, :], AF.Sigmoid)

        for s in range(N_SLABS):
            a = s % A
            slab = slabs.tile([P, RB, NCH], dt.float16)
            scr = scrs.tile([P, RB, 2], dt.float16)
            nc.sync.dma_start(out=slab[:, :, :], in_=xe_ap[s, :, :])
            # sigmoid(-t) for tw/th BEFORE cols 2:4 are overwritten in place
            nc.scalar.activation(
                scr[:, :, :], slab[:, :, 2:4], AF.Sigmoid, scale=-1.0
            )
            nc.scalar.activation(slab[:, :, 0:4], slab[:, :, 0:4], AF.Sigmoid)
            nc.scalar.activation(slab[:, :, 4:NCH], slab[:, :, 4:NCH], AF.Sigmoid)
            with nc.allow_low_precision(reason="fp16 decode, 2e-2 tolerance"):
                nc.vector.reciprocal(scr[:, :, :], scr[:, :, :])
                # cols 0:4 *= [8, 8, aw, ah]
                nc.vector.tensor_tensor(
                    slab[:, :, 0:4], slab[:, :, 0:4], kmul[a][:, :, :], ALU.mult
                )
                # cols 2:4 *= 1/sigmoid(-t)  ->  aw * exp(tw), ah * exp(th)
                nc.vector.tensor_tensor(
                    slab[:, :, 2:4], slab[:, :, 2:4], scr[:, :, :], ALU.mult
                )
                # cols 0:2 += [8*cx, 8*cy]
                nc.vector.tensor_tensor(
                    slab[:, :, 0:2], slab[:, :, 0:2], kadd[:, :, :], ALU.add
                )
            nc.scalar.dma_start(out=out_ap[s, :, :], in_=slab[:, :, :])

    nc.compile()
    _CACHE["nc"] = nc
    return nc


def _pack_core_input(x_core):
    """x_core [B_PER_CORE, 255, 52, 52] f32 -> xe [N_SLABS, P, FREE] fp16."""
    xr = x_core.reshape(B_PER_CORE, A, NCH, GG)
    xt = xr.transpose(0, 1, 3, 2)  # [b, a, grid, ch]
    xe = np.zeros((B_PER_CORE, A, ROWS_PAD, NCH), dtype=np.float16)
    xe[:, :, :GG, :] = xt
    return xe.reshape(N_SLABS, P, FREE)


def kernel(x):
    x = np.ascontiguousarray(np.asarray(x), dtype=np.float32)
    assert x.shape == (B, A * NCH, G, G), x.shape
    nc = build_nc()
    from concourse.bass_utils import run_bass_kernel_spmd

    kc = _build_consts()
    in_maps = []
    for c in range(N_CORES):
        in_maps.append(
            {
                "xe": _pack_core_input(x[c * B_PER_CORE : (c + 1) * B_PER_CORE]),
                "kc": kc,
            }
        )
    # transient NRT_EXEC_UNIT_UNRECOVERABLE has been observed once on a cold
    # first execution and never again; retry a couple of times before failing
    for attempt in range(3):
        try:
            res = run_bass_kernel_spmd(nc, in_maps, core_ids=list(range(N_CORES)))
            break
        except Exception:  # noqa: BLE001
            if attempt == 2:
                raise
            import time

            time.sleep(2.0 * (attempt + 1))
    _CACHE["last_res"] = res
    full = np.stack([r["out"] for r in res.results], axis=0)  # [8, 24, P, FREE] f16
    full = full.reshape(N_CORES, B_PER_CORE, A, ROWS_PAD, NCH)[:, :, :, :GG, :]
    return np.ascontiguousarray(full.astype(np.float32)).reshape(B, A * GG, NCH)


# revision 4
# speedup vs baseline: 2.0233x; 1.0342x over previous
"""YOLO-style detection head decode on 8 Trainium2 NeuronCores.

Input : x [64, 255, 52, 52] f32
Output: [64, 8112, 85] f32  (bbox(4) | conf(1) | cls(80), sigmoid/exp decoded)

Strategy (pure data parallel, 8 batches per core; fp16 transfer dtype —
the 2e-2 rel-err budget admits ~5e-3 worst-case fp16 decode error):
  - host transposes each (batch, anchor) slab to grid-major [2704, 85]
    (channel order tx,ty,tw,th,conf,cls already matches the output), pads
    rows 2704->2706 and tiles as [123 partitions, 22 rows x 85 ch] fp16.
    The device then needs NO transpose: no PE matmul, no PSUM drain.
  - device per slab: one 123-descriptor load (3740 B/descriptor), three
    in-place ACT sigmoids (cols 0:4, cols 2:4 with scale=-1 into scratch,
    cols 4:85), tiny DVE fixups, one 123-descriptor store.
  - exp(t) is computed as sigmoid(t)/sigmoid(-t) so the ACT engine only
    ever needs the sigmoid table: no per-slab 1283 ns ACT-table reloads
    (no hw table set holds both Sigmoid and Exp).
  - box decode folded into two constant tiles: out[0:4] *= [8,8,aw,ah]
    (stride scale + anchor scale), out[0:2] += [8*cx, 8*cy].
  - everything is elementwise in-place on one tile, so DMA traffic is the
    bare input+output (2 x 11.04 MB/core fp16): the shared DMA engine pool
    (360 GB/s) is the roofline at ~62 us; ACT sits at ~47 us under it.
"""

import numpy as np

G = 52
GG = G * G  # 2704
A = 3
NCH = 85  # 5 + 80
B = 64
N_CORES = 8
B_PER_CORE = B // N_CORES  # 8
STRIDE = 8.0  # 416 / 52
ANCHORS_PX = np.array([[10.0, 13.0], [16.0, 30.0], [33.0, 23.0]], dtype=np.float64)
P = 123  # partitions per slab tile
RB = 22  # grid rows per partition
ROWS_PAD = P * RB  # 2706
FREE = RB * NCH  # 1870
N_SLABS = B_PER_CORE * A  # 24
KC_W = A * RB * 4 + RB * 2  # 3 kmul tiles + kadd = 308

_CACHE = {}


def _build_consts():
    """kc [P, KC_W] fp16: per-anchor [8,8,aw,ah] mul tiles + [8cx,8cy] add."""
    g = np.arange(ROWS_PAD, dtype=np.float64)
    cx8 = (STRIDE * (g % G)).reshape(P, RB)
    cy8 = (STRIDE * ((g // G) % G)).reshape(P, RB)  # pad rows wrap; sliced off
    kadd = np.stack([cx8, cy8], axis=-1)  # [P, RB, 2]
    km = np.zeros((A, P, RB, 4), dtype=np.float64)
    km[..., 0] = STRIDE
    km[..., 1] = STRIDE
    for a in range(A):
        km[a, ..., 2] = ANCHORS_PX[a, 0]
        km[a, ..., 3] = ANCHORS_PX[a, 1]
    kc = np.concatenate(
        [km.transpose(1, 0, 2, 3).reshape(P, A * RB * 4), kadd.reshape(P, RB * 2)],
        axis=1,
    )
    return kc.astype(np.float16)


def build_nc():
    if "nc" in _CACHE:
        return _CACHE["nc"]
    from contextlib import ExitStack

    import concourse.bacc as bacc
    import concourse.tile as tile
    from concourse import mybir

    AF = mybir.ActivationFunctionType
    ALU = mybir.AluOpType
    dt = mybir.dt

    nc = bacc.Bacc("TRN2", target_bir_lowering=False, debug=False)
    xe_t = nc.dram_tensor("xe", [N_SLABS, P, FREE], dt.float16, kind="ExternalInput")
    kc_t = nc.dram_tensor("kc", [P, KC_W], dt.float16, kind="ExternalInput")
    out_t = nc.dram_tensor("out", [N_SLABS, P, FREE], dt.float16, kind="ExternalOutput")
    xe_ap = xe_t.ap()
    kc_ap = kc_t.ap()
    out_ap = out_t.ap()

    with ExitStack() as ctx:
        tc = ctx.enter_context(tile.TileContext(nc))
        singles = ctx.enter_context(tc.tile_pool(name="singles", bufs=1))
        slabs = ctx.enter_context(tc.tile_pool(name="slabs", bufs=10))
        scrs = ctx.enter_context(tc.tile_pool(name="scrs", bufs=10))

        # preload the sigmoid table once; every activation below is Sigmoid,
        # so the ACT engine never reloads a table mid-stream
        dummy = singles.tile([1, 1], dt.float32)
        nc.vector.memset(dummy[:, :], 0.0)
        nc.scalar.activation(dummy[:, :], dummy[:, :], AF.Sigmoid)

        # work list: (slab, block_lo, block_hi). The final slab is split
        # along the free dim so the drain tail ends with a short store (the
        # last transfer is 6/22 of a slab) instead of a full 1278ns one.
        pieces = [(s, 0, RB) for s in range(N_SLABS - 1)]
        pieces += [(N_SLABS - 1, 0, 16), (N_SLABS - 1, 16, RB)]

        # issue the first loads ahead of the constants so the first big
        # transfer starts as early as the DMA latency chain allows
        piece_tiles = {}
        for i in range(min(2, len(pieces))):
            s, b0, b1 = pieces[i]
            slab = slabs.tile([P, b1 - b0, NCH], dt.float16)
            nc.sync.dma_start(
                out=slab[:, :, :], in_=xe_ap[s, :, b0 * NCH : b1 * NCH]
            )
            piece_tiles[i] = slab

        kc = singles.tile([P, KC_W], dt.float16)
        nc.sync.dma_start(out=kc[:, :], in_=kc_ap[:, :])
        kmul = [
            kc[:, a * RB * 4 : (a + 1) * RB * 4].rearrange("p (t c) -> p t c", c=4)
            for a in range(A)
        ]
        kadd = kc[:, A * RB * 4 :].rearrange("p (t c) -> p t c", c=2)

        for i, (s, b0, b1) in enumerate(pieces):
            a = s % A
            last = i >= len(pieces) - 2
            if i in piece_tiles:
                slab = piece_tiles[i]
            else:
                slab = slabs.tile([P, b1 - b0, NCH], dt.float16)
                nc.sync.dma_start(
                    out=slab[:, :, :], in_=xe_ap[s, :, b0 * NCH : b1 * NCH]
                )
            scr = scrs.tile([P, b1 - b0, 2], dt.float16)
            # sigmoid(-t) for tw/th BEFORE cols 2:4 are overwritten in place
            nc.scalar.activation(
                scr[:, :, :], slab[:, :, 2:4], AF.Sigmoid, scale=-1.0
            )
            nc.scalar.activation(slab[:, :, :], slab[:, :, :], AF.Sigmoid)
            with nc.allow_low_precision(reason="fp16 decode, 2e-2 tolerance"):
                nc.vector.reciprocal(scr[:, :, :], scr[:, :, :])
                # cols 0:4 *= [8, 8, aw, ah]
                nc.vector.tensor_tensor(
                    slab[:, :, 0:4], slab[:, :, 0:4], kmul[a][:, b0:b1, :], ALU.mult
                )
                # cols 2:4 *= 1/sigmoid(-t)  ->  aw * exp(tw), ah * exp(th)
                nc.vector.tensor_tensor(
                    slab[:, :, 2:4], slab[:, :, 2:4], scr[:, :, :], ALU.mult
                )
                # cols 0:2 += [8*cx, 8*cy]
                nc.vector.tensor_tensor(
                    slab[:, :, 0:2], slab[:, :, 0:2], kadd[:, b0:b1, :], ALU.add
                )
            # steady state: store via the otherwise-idle gpsimd SWDGE path,
            # keeping the ACT sequencer free of 630ns HWDGE holds. Final
            # pieces: SP HWDGE (625ns) beats Pool DGE (1036ns) on the drain
            # critical path.
            eng = nc.sync if last else nc.gpsimd
            eng.dma_start(
                out=out_ap[s, :, b0 * NCH : b1 * NCH], in_=slab[:, :, :]
            )

    nc.compile()
    _CACHE["nc"] = nc
    return nc


def _pack_core_input(x_core):
    """x_core [B_PER_CORE, 255, 52, 52] f32 -> xe [N_SLABS, P, FREE] fp16."""
    xr = x_core.reshape(B_PER_CORE, A, NCH, GG)
    xt = xr.transpose(0, 1, 3, 2)  # [b, a, grid, ch]
    xe = np.zeros((B_PER_CORE, A, ROWS_PAD, NCH), dtype=np.float16)
    xe[:, :, :GG, :] = xt
    return xe.reshape(N_SLABS, P, FREE)


def kernel(x):
    x = np.ascontiguousarray(np.asarray(x), dtype=np.float32)
    assert x.shape == (B, A * NCH, G, G), x.shape
    nc = build_nc()
    from concourse.bass_utils import run_bass_kernel_spmd

    kc = _build_consts()
    in_maps = []
    for c in range(N_CORES):
        in_maps.append(
            {
                "xe": _pack_core_input(x[c * B_PER_CORE : (c + 1) * B_PER_CORE]),
                "kc": kc,
            }
        )
    # transient NRT_EXEC_UNIT_UNRECOVERABLE has been observed once on a cold
    # first execution and never again; retry a couple of times before failing
    for attempt in range(3):
        try:
            res = run_bass_kernel_spmd(nc, in_maps, core_ids=list(range(N_CORES)))
            break
        except Exception:  # noqa: BLE001
            if attempt == 2:
                raise
            import time

            time.sleep(2.0 * (attempt + 1))
    _CACHE["last_res"] = res
    full = np.stack([r["out"] for r in res.results], axis=0)  # [8, 24, P, FREE] f16
    full = full.reshape(N_CORES, B_PER_CORE, A, ROWS_PAD, NCH)[:, :, :, :GG, :]
    return np.ascontiguousarray(full.astype(np.float32)).reshape(B, A * GG, NCH)


# revision 7
# speedup vs baseline: 2.0280x; 1.0023x over previous
"""YOLO-style detection head decode on 8 Trainium2 NeuronCores.

Input : x [64, 255, 52, 52] f32
Output: [64, 8112, 85] f32  (bbox(4) | conf(1) | cls(80), sigmoid/exp decoded)

Strategy (pure data parallel, 8 batches per core; fp16 transfer dtype —
the 2e-2 rel-err budget admits ~5e-3 worst-case fp16 decode error):
  - host transposes each (batch, anchor) slab to grid-major [2704, 85]
    (channel order tx,ty,tw,th,conf,cls already matches the output), pads
    rows 2704->2706 and tiles as [123 partitions, 22 rows x 85 ch] fp16.
    The device then needs NO transpose: no PE matmul, no PSUM drain.
  - device per slab: one 123-descriptor load (3740 B/descriptor), three
    in-place ACT sigmoids (cols 0:4, cols 2:4 with scale=-1 into scratch,
    cols 4:85), tiny DVE fixups, one 123-descriptor store.
  - exp(t) is computed as sigmoid(t)/sigmoid(-t) so the ACT engine only
    ever needs the sigmoid table: no per-slab 1283 ns ACT-table reloads
    (no hw table set holds both Sigmoid and Exp).
  - box decode folded into two constant tiles: out[0:4] *= [8,8,aw,ah]
    (stride scale + anchor scale), out[0:2] += [8*cx, 8*cy].
  - everything is elementwise in-place on one tile, so DMA traffic is the
    bare input+output (2 x 11.04 MB/core fp16): the shared DMA engine pool
    (360 GB/s) is the roofline at ~62 us; ACT sits at ~47 us under it.
"""

import numpy as np

G = 52
GG = G * G  # 2704
A = 3
NCH = 85  # 5 + 80
B = 64
N_CORES = 8
B_PER_CORE = B // N_CORES  # 8
STRIDE = 8.0  # 416 / 52
ANCHORS_PX = np.array([[10.0, 13.0], [16.0, 30.0], [33.0, 23.0]], dtype=np.float64)
P = 123  # partitions per slab tile
RB = 22  # grid rows per partition
ROWS_PAD = P * RB  # 2706
FREE = RB * NCH  # 1870
N_SLABS = B_PER_CORE * A  # 24
KC_W = A * RB * 4 + RB * 2  # 3 kmul tiles + kadd = 308

_CACHE = {}


def build_nc():
    if "nc" in _CACHE:
        return _CACHE["nc"]
    from contextlib import ExitStack

    import concourse.bacc as bacc
    import concourse.tile as tile
    from concourse import mybir

    AF = mybir.ActivationFunctionType
    ALU = mybir.AluOpType
    dt = mybir.dt

    nc = bacc.Bacc("TRN2", target_bir_lowering=False, debug=False)
    xe_t = nc.dram_tensor("xe", [N_SLABS, P, FREE], dt.float16, kind="ExternalInput")
    ka_t = nc.dram_tensor("ka", [P, RB * 2], dt.float16, kind="ExternalInput")
    out_t = nc.dram_tensor("out", [N_SLABS, P, FREE], dt.float16, kind="ExternalOutput")
    xe_ap = xe_t.ap()
    ka_ap = ka_t.ap()
    out_ap = out_t.ap()

    with ExitStack() as ctx:
        tc = ctx.enter_context(tile.TileContext(nc))
        singles = ctx.enter_context(tc.tile_pool(name="singles", bufs=1))
        slabs = ctx.enter_context(tc.tile_pool(name="slabs", bufs=10))
        scrs = ctx.enter_context(tc.tile_pool(name="scrs", bufs=10))

        # preload the sigmoid table once; every activation below is Sigmoid,
        # so the ACT engine never reloads a table mid-stream
        dummy = singles.tile([1, 1], dt.float32)
        nc.vector.memset(dummy[:, :], 0.0)
        nc.scalar.activation(dummy[:, :], dummy[:, :], AF.Sigmoid)

        # work list: (slab, block_lo, block_hi). The final slab is split
        # along the free dim so the drain tail ends with a short store (the
        # last transfer is 6/22 of a slab) instead of a full 1278ns one.
        pieces = [(s, 0, RB) for s in range(N_SLABS - 1)]
        pieces += [(N_SLABS - 1, 0, 16), (N_SLABS - 1, 16, RB)]

        # issue the first loads ahead of the constants so the first big
        # transfer starts as early as the DMA latency chain allows
        piece_tiles = {}
        for i in range(min(2, len(pieces))):
            s, b0, b1 = pieces[i]
            slab = slabs.tile([P, b1 - b0, NCH], dt.float16)
            nc.sync.dma_start(
                out=slab[:, :, :], in_=xe_ap[s, :, b0 * NCH : b1 * NCH]
            )
            piece_tiles[i] = slab

        # kmul is memset-generated on the idle-early gpsimd engine; only the
        # non-affine [8cx, 8cy] table rides a (60ns) DMA. Every DMA byte is
        # on the shared-engine-pool critical path, early engine cycles are free.
        kc = singles.tile([P, KC_W], dt.float16)
        kmul = [
            kc[:, a * RB * 4 : (a + 1) * RB * 4].rearrange("p (t c) -> p t c", c=4)
            for a in range(A)
        ]
        kadd = kc[:, A * RB * 4 :].rearrange("p (t c) -> p t c", c=2)
        nc.sync.dma_start(out=kadd[:, :, :], in_=ka_ap[:, :])
        for a in range(A):
            nc.gpsimd.memset(kmul[a][:, :, 0:2], STRIDE)  # x,y stride scale
            nc.gpsimd.memset(kmul[a][:, :, 2:3], float(ANCHORS_PX[a, 0]))
            nc.gpsimd.memset(kmul[a][:, :, 3:4], float(ANCHORS_PX[a, 1]))

        for i, (s, b0, b1) in enumerate(pieces):
            a = s % A
            last = i >= len(pieces) - 2
            if i in piece_tiles:
                slab = piece_tiles[i]
            else:
                slab = slabs.tile([P, b1 - b0, NCH], dt.float16)
                nc.sync.dma_start(
                    out=slab[:, :, :], in_=xe_ap[s, :, b0 * NCH : b1 * NCH]
                )
            scr = scrs.tile([P, b1 - b0, 2], dt.float16)
            # sigmoid(-t) for tw/th BEFORE cols 2:4 are overwritten in place
            nc.scalar.activation(
                scr[:, :, :], slab[:, :, 2:4], AF.Sigmoid, scale=-1.0
            )
            nc.scalar.activation(slab[:, :, :], slab[:, :, :], AF.Sigmoid)
            with nc.allow_low_precision(reason="fp16 decode, 2e-2 tolerance"):
                nc.vector.reciprocal(scr[:, :, :], scr[:, :, :])
                # cols 0:4 *= [8, 8, aw, ah]
                nc.vector.tensor_tensor(
                    slab[:, :, 0:4], slab[:, :, 0:4], kmul[a][:, b0:b1, :], ALU.mult
                )
                # cols 2:4 *= 1/sigmoid(-t)  ->  aw * exp(tw), ah * exp(th)
                nc.vector.tensor_tensor(
                    slab[:, :, 2:4], slab[:, :, 2:4], scr[:, :, :], ALU.mult
                )
                # cols 0:2 += [8*cx, 8*cy]
                nc.vector.tensor_tensor(
                    slab[:, :, 0:2], slab[:, :, 0:2], kadd[:, b0:b1, :], ALU.add
                )
            # steady state: store via the otherwise-idle gpsimd SWDGE path,
            # keeping the ACT sequencer free of 630ns HWDGE holds. Final
            # pieces: SP HWDGE (625ns) beats Pool DGE (1036ns) on the drain
            # critical path.
            eng = nc.sync if last else nc.gpsimd
            eng.dma_start(
                out=out_ap[s, :, b0 * NCH : b1 * NCH], in_=slab[:, :, :]
            )

    nc.compile()
    _CACHE["nc"] = nc
    return nc


def _build_kadd():
    g = np.arange(ROWS_PAD, dtype=np.float64)
    cx8 = (STRIDE * (g % G)).reshape(P, RB)
    cy8 = (STRIDE * ((g // G) % G)).reshape(P, RB)  # pad rows wrap; sliced off
    return np.stack([cx8, cy8], axis=-1).reshape(P, RB * 2).astype(np.float16)


def _pack_core_input(x_core):
    """x_core [B_PER_CORE, 255, 52, 52] f32 -> xe [N_SLABS, P, FREE] fp16."""
    xr = x_core.reshape(B_PER_CORE, A, NCH, GG)
    xt = xr.transpose(0, 1, 3, 2)  # [b, a, grid, ch]
    xe = np.zeros((B_PER_CORE, A, ROWS_PAD, NCH), dtype=np.float16)
    xe[:, :, :GG, :] = xt
    return xe.reshape(N_SLABS, P, FREE)


def kernel(x):
    x = np.ascontiguousarray(np.asarray(x), dtype=np.float32)
    assert x.shape == (B, A * NCH, G, G), x.shape
    nc = build_nc()
    from concourse.bass_utils import run_bass_kernel_spmd

    ka = _build_kadd()
    in_maps = []
    for c in range(N_CORES):
        in_maps.append(
            {
                "xe": _pack_core_input(x[c * B_PER_CORE : (c + 1) * B_PER_CORE]),
                "ka": ka,
            }
        )
    # transient NRT_EXEC_UNIT_UNRECOVERABLE has been observed once on a cold
    # first execution and never again; retry a couple of times before failing
    for attempt in range(3):
        try:
            res = run_bass_kernel_spmd(nc, in_maps, core_ids=list(range(N_CORES)))
            break
        except Exception:  # noqa: BLE001
            if attempt == 2:
                raise
            import time

            time.sleep(2.0 * (attempt + 1))
    _CACHE["last_res"] = res
    full = np.stack([r["out"] for r in res.results], axis=0)  # [8, 24, P, FREE] f16
    full = full.reshape(N_CORES, B_PER_CORE, A, ROWS_PAD, NCH)[:, :, :, :GG, :]
    return np.ascontiguousarray(full.astype(np.float32)).reshape(B, A * GG, NCH)
